# revision 49
# baseline (speedup 1.0000x reference)
"""Antialiased 2x upsampling (StyleGAN2 upsample_2d, k=[1,3,3,1], factor=2).

Input  x: (8, 256, 256, 64) f32 NHWC  ->  output: (8, 511, 511, 64) f32.

Math (separable, polyphase; all taps exact in f32 up to one 1/3 rounding):
  A[i] = x[i-1] (zero-padded), B[i] = x[i]
  g = A/3 + B        (even out rows 2i,   scale deficit absorbed below)
  h = B/3 + A        (odd  out rows 2i-1)
  out[2i,   2j]   = 9/16*g[j]   + 3/16*g[j-1]
  out[2i,   2j-1] = 9/16*g[j-1] + 3/16*g[j]
  out[2i-1, 2j]   = 9/16*h[j]   + 3/16*h[j-1]
  out[2i-1, 2j-1] = 9/16*h[j-1] + 3/16*h[j]

Sharding: pure data parallel, one batch image per NeuronCore (8 cores).
Layout: partition dim = input row i, free dim = w*C+c. All shifts are
free-dim AP offsets except the H-shift, realized by loading a row-shifted
second copy (A) of each input tile from DRAM.

Performance notes (measured on TRN2):
- SBUF tiles are bf16 (DMA casts f32<->bf16 in flight); HBM traffic stays
  f32 on both ends. Halves SBUF footprint -> WT=64 tiles, bigger DMA
  descriptors, and 2x-mode DVE adds. rel err ~3e-3 (gate is 2e-2).
- All bulk DMAs go through gpsimd (SWDGE, required for cast) and are split
  into 64-partition chunks: descriptor packets then spread across all 16
  SDMA engines and loads interleave with stores in the descriptor rings.
- Loads are issued PRE iterations ahead of compute so the store-emission
  waits on the single SWDGE queue never starve the load stream.
- W-pass = tensor_tensor adds of two pre-scaled (ACT) copies: plain adds
  hit the DVE 2x bf16 packing mode; scalar_tensor_tensor does not.
- The tiny row-0 passes are scattered through the main loop so they hide
  under the DMA-saturated steady state.
"""

import numpy as np

import concourse.bacc as bacc
import concourse.mybir as mybir
from concourse.tile import TileContext
from concourse.bass_utils import run_bass_kernel_spmd

F32 = mybir.dt.float32
BF16 = mybir.dt.bfloat16
MULT = mybir.AluOpType.mult
ADD = mybir.AluOpType.add

B_FULL, H_FULL, W_FULL, C_FULL = 8, 256, 256, 64
N_CORES = 8


def build_upsample_tile(tc, out, x, H, W, C, P, WT, SBDT=BF16):
    """Trace the upsampling kernel into TileContext tc.

    x:   DRAM AP [H, W*C]
    out: DRAM AP [2H-1, (2W-1)*C]
    P:   partition tile height (input rows per tile)
    WT:  input cols per w-tile
    """
    nc = tc.nc
    assert W % WT == 0
    n_wt = W // WT
    FW = (WT + 1) * C  # tile free width: cols w0-1 .. w0+WT-1

    # h-tiles cover input rows i = i0 .. i0+PT-1 (partition p <-> i = i0+p).
    # Row i produces out rows 2i-1 (odd) and 2i (even). i=0 is handled by a
    # separate 1-partition pass (out row 0 only) so that the shifted A-load
    # (src rows i0-1..) never reads row -1 and all SBUF APs start at
    # partition 0 (hardware allows starts only at 0/32/64/96).
    h_tiles = []
    i0 = 1
    while i0 < H:
        h_tiles.append((i0, min(P, H - i0)))
        i0 += P

    seg = 2 * WT * C  # one output row segment (2*WT cols)

    with (
        tc.tile_pool(name="io", bufs=2) as io_pool,
        tc.tile_pool(name="mid", bufs=1) as mid_pool,
        tc.tile_pool(name="rb", bufs=2) as rb_pool,
    ):
        def v(t, qlo, PT):
            return t[:PT, qlo * C : (qlo + WT) * C].rearrange("p (j c) -> p j c", c=C)

        def wpass(f9, f3, rbv, s, PT):
            # out[r, 2j]   = f9[j]   + f3[j-1]   (even cols -> q=1 slot)
            # plain tensor_tensor adds of pre-scaled copies: eligible for the
            # DVE 2x bf16 packing mode (scalar_tensor_tensor is not)
            nc.vector.tensor_add(
                out=rbv[:PT, s, :, 1, :], in0=v(f9, 1, PT), in1=v(f3, 0, PT)
            )
            # out[r, 2j-1] = f9[j-1] + f3[j]     (odd cols -> q=0 slot)
            nc.vector.tensor_add(
                out=rbv[:PT, s, :, 0, :], in0=v(f9, 0, PT), in1=v(f3, 1, PT)
            )

        def wparams(wt):
            w0 = wt * WT
            return dict(
                w0=w0,
                cl=(w0 - 1) * C,
                skip=C if w0 == 0 else 0,
                dcol_lo=0 if w0 == 0 else (2 * w0 - 1) * C,
                dw=seg - (C if w0 == 0 else 0),
                ld_w=WT * C if w0 == 0 else FW,
                ld_off=C if w0 == 0 else 0,
            )

        def pchunks(PT):
            # legal SBUF partition starts are 0/32/64/96; 64-partition chunks
            # measured fastest (ring interleave vs Q7 emission count)
            return [(q0, min(q0 + 64, PT)) for q0 in (0, 64) if q0 < PT]

        # --- row 0 pass (tiny): out[0] = W-upsample of x[0] (x[-1] = 0).
        # Scattered into the main loop so it hides under the DMA-saturated
        # steady state instead of serializing at the start or end.
        def row0_pass(wt):
            p = wparams(wt)
            B0 = io_pool.tile([1, FW], SBDT, tag="AB", name=f"B0_{wt}")
            if p["w0"] == 0:
                nc.vector.memset(B0[:, 0:C], 0.0)
            nc.gpsimd.dma_start(
                out=B0[:, p["ld_off"] : p["ld_off"] + p["ld_w"]],
                in_=x[0:1, p["cl"] + p["ld_off"] : p["cl"] + FW],
            )
            g30 = mid_pool.tile([1, FW], SBDT, tag="g30", bufs=1, name=f"g30_{wt}")
            nc.scalar.mul(g30[:], B0[:], 3.0 / 16.0)
            g90 = mid_pool.tile([1, FW], SBDT, tag="g90", bufs=1, name=f"g90_{wt}")
            nc.scalar.mul(g90[:], B0[:], 9.0 / 16.0)
            rb0 = rb_pool.tile([1, 2 * WT * C], SBDT, tag="rb0", bufs=1, name=f"rb0_{wt}")
            rbv0 = rb0.rearrange("p (j q c) -> p j q c", j=WT, q=2, c=C)
            nc.vector.tensor_add(
                out=rbv0[:1, :, 1, :], in0=v(g90, 1, 1), in1=v(g30, 0, 1)
            )
            nc.vector.tensor_add(
                out=rbv0[:1, :, 0, :], in0=v(g90, 0, 1), in1=v(g30, 1, 1)
            )
            return rb0, p

        def row0_store(rb0, p):
            # emitted one step after row0_pass: by then its wpass has long
            # retired, so this store's wait never stalls the gpsimd queue
            nc.gpsimd.dma_start(
                out=out[0:1, p["dcol_lo"] : p["dcol_lo"] + p["dw"]],
                in_=rb0[:1, p["skip"] : seg],
            )

        # --- main tiles, software-pipelined: loads issued PRE iterations
        # ahead of compute so the gpsimd queue's wait-for-compute (before
        # each store emission) never blocks the next loads.
        steps = [(ti, wt) for wt in range(n_wt) for ti in range(len(h_tiles))]
        N = len(steps)
        PRE = 2
        tiles = {}

        def load(s):
            ti, wt = steps[s]
            i0, PT = h_tiles[ti]
            p = wparams(wt)
            lo, lw = p["ld_off"], p["ld_w"]
            # A[q] = x[i0+q-1], B[q] = x[i0+q]; split into 64-partition DMAs
            # so concurrent one-packet transfers spread across SDMA engines.
            A = io_pool.tile([PT, FW], SBDT, tag="A", name=f"A_{ti}_{wt}")
            Bt = io_pool.tile([PT, FW], SBDT, tag="B", name=f"B_{ti}_{wt}")
            if p["w0"] == 0:
                nc.vector.memset(A[:PT, 0:C], 0.0)
                nc.vector.memset(Bt[:PT, 0:C], 0.0)
            for q0, q1 in pchunks(PT):
                nc.gpsimd.dma_start(
                    out=A[q0:q1, lo : lo + lw],
                    in_=x[i0 - 1 + q0 : i0 - 1 + q1,
                          p["cl"] + lo : p["cl"] + lo + lw],
                )
            for q0, q1 in pchunks(PT):
                nc.gpsimd.dma_start(
                    out=Bt[q0:q1, lo : lo + lw],
                    in_=x[i0 + q0 : i0 + q1, p["cl"] + lo : p["cl"] + lo + lw],
                )
            tiles[s] = (A, Bt)

        def compute_store(s):
            ti, wt = steps[s]
            i0, PT = h_tiles[ti]
            p = wparams(wt)
            A, Bt = tiles.pop(s)
            A = A[:PT, :]
            Bt = Bt[:PT, :]

            g = mid_pool.tile([PT, FW], SBDT, tag="g", name=f"g_{ti}_{wt}")
            hh = mid_pool.tile([PT, FW], SBDT, tag="h", name=f"h_{ti}_{wt}")
            g3 = mid_pool.tile([PT, FW], SBDT, tag="g3", name=f"g3_{ti}_{wt}")
            h3 = mid_pool.tile([PT, FW], SBDT, tag="h3", name=f"h3_{ti}_{wt}")
            nc.vector.scalar_tensor_tensor(
                out=g[:], in0=A, scalar=1.0 / 3.0, in1=Bt, op0=MULT, op1=ADD
            )
            nc.vector.scalar_tensor_tensor(
                out=hh[:], in0=Bt, scalar=1.0 / 3.0, in1=A, op0=MULT, op1=ADD
            )
            g9 = mid_pool.tile([PT, FW], SBDT, tag="g9", name=f"g9_{ti}_{wt}")
            h9 = mid_pool.tile([PT, FW], SBDT, tag="h9", name=f"h9_{ti}_{wt}")
            nc.scalar.mul(g3[:], g[:], 3.0 / 16.0)
            nc.scalar.mul(g9[:], g[:], 9.0 / 16.0)
            nc.scalar.mul(h3[:], hh[:], 3.0 / 16.0)
            nc.scalar.mul(h9[:], hh[:], 9.0 / 16.0)

            # rowbuf: [odd-row seg | even-row seg] so DRAM rows ascend;
            # each seg = WT x [oddcol | evencol] x C
            rb = rb_pool.tile([PT, 4 * WT * C], SBDT, tag="rb", name=f"rb_{ti}_{wt}")
            rbv = rb.rearrange("p (s j q c) -> p s j q c", s=2, j=WT, q=2, c=C)
            wpass(h9, h3, rbv, 0, PT)  # odd rows 2i-1 -> first segment
            wpass(g9, g3, rbv, 1, PT)  # even rows 2i -> second segment

            # stores: odd rows 2(i0+q)-1 and even rows 2(i0+q), split into
            # 64-partition one-packet DMAs like the loads
            for q0, q1 in pchunks(PT):
                r0 = 2 * (i0 + q0) - 1
                nc.gpsimd.dma_start(
                    out=out[r0 : r0 + 2 * (q1 - q0) - 1 : 2,
                            p["dcol_lo"] : p["dcol_lo"] + p["dw"]],
                    in_=rb[q0:q1, p["skip"] : seg],
                )
            for q0, q1 in pchunks(PT):
                r0 = 2 * (i0 + q0)
                nc.gpsimd.dma_start(
                    out=out[r0 : r0 + 2 * (q1 - q0) - 1 : 2,
                            p["dcol_lo"] : p["dcol_lo"] + p["dw"]],
                    in_=rb[q0:q1, seg + p["skip"] : 2 * seg],
                )

        pending_row0 = None
        for s in range(N + PRE):
            if s < N:
                load(s)
            if s >= PRE:
                compute_store(s - PRE)
                if pending_row0 is not None:
                    row0_store(*pending_row0)
                    pending_row0 = None
                # scatter the n_wt tiny row-0 passes across mid-loop steps
                k = s - PRE
                if 1 <= k <= n_wt:
                    pending_row0 = row0_pass(k - 1)
        if pending_row0 is not None:
            row0_store(*pending_row0)


def build_nc(H=H_FULL, W=W_FULL, C=C_FULL, P=128, WT=64):
    nc = bacc.Bacc(
        "TRN2", target_bir_lowering=False, debug=False,
        dynamic_dma_scratch_size=16384,
    )
    x = nc.declare_dram_parameter("x", [H, W * C], F32, isOutput=False).ap()
    out = nc.declare_dram_parameter(
        "out", [2 * H - 1, (2 * W - 1) * C], F32, isOutput=True
    ).ap()
    with TileContext(nc) as tc:
        build_upsample_tile(tc, out, x, H, W, C, P, WT, SBDT=BF16)
    nc.compile()
    return nc


_NC_CACHE = {}


def _get_nc():
    key = (H_FULL, W_FULL, C_FULL)
    if key not in _NC_CACHE:
        _NC_CACHE[key] = build_nc()
    return _NC_CACHE[key]


def run_spmd(x, trace=False, **kwargs):
    """x: (8, 256, 256, 64) f32. Returns (BassKernelResults, out (8,511,511,64))."""
    nc = _get_nc()
    in_maps = [
        {"x": np.ascontiguousarray(x[b]).reshape(H_FULL, W_FULL * C_FULL)}
        for b in range(N_CORES)
    ]
    res = run_bass_kernel_spmd(
        nc, in_maps, core_ids=list(range(N_CORES)), trace=trace, **kwargs
    )
    out = np.stack(
        [
            res.results[b]["out"].reshape(2 * H_FULL - 1, 2 * W_FULL - 1, C_FULL)
            for b in range(N_CORES)
        ]
    )
    return res, out


def kernel(x):
    x = np.asarray(x, dtype=np.float32)
    _, out = run_spmd(x, trace=False)
    return out
